# revision 27
# baseline (speedup 1.0000x reference)
"""Multi-head attention TRN2 Bass kernel (v2, all-bf16).

Problem: B=4, S=2048, D=1024, H=16 heads (DK=64), fp32 reference, random
0/1 attention mask broadcast over heads.

Sharding: 8 cores = (batch b, query-half) pairs.  Core c handles batch
c//2, query rows [(c%2)*1024, (c%2+1)*1024).  K/V projections for the
batch are computed redundantly on the 2 cores sharing a batch; no
collectives, each core writes a disjoint output slice.

v2 design (vs v1 fp32r baseline):
  - Everything bf16: host pre-casts inputs/weights (W_q, b_q pre-scaled
    by 1/sqrt(DK) so no on-device scaling), intermediates bf16.  Matmul
    throughput is the same 1 col/cycle as fp32r, but bf16 halves SBUF/
    DMA, enables fast weight loads, N=1024 moving operands, and DVE
    2x mode for the mask multiply.
  - No DRAM spills: qhT/khT/vh/mask/ct all SBUF-resident (bf16 fits).
  - Scores matmuls for the two heads of a pair auto-pack into PE array
    row-halves (K=DK=64, tile_position (0,0)/(64,0) derived from base
    partitions), so consecutive head matmuls can overlap.
  - exp on ACT (PSUM fp32 -> SBUF bf16); this is the true bottleneck
    (~293us of ACT time per core); everything else overlaps it.
  - mask applied multiplicatively on DVE in bf16 2x mode.
  - AV matmul with 65-wide lhsT ([vh_h | 1]): row 64 accumulates the
    softmax denominators for free.
  - Normalization: denominators reciprocal'd once per head-pair,
    DMA-broadcast via DRAM round trip, one [128, SQ] bf16 2x multiply.
  - PSUM budget exactly 8 banks: scores 2x[128,1024]f32 (4) +
    attention accum [65,1024]f32 (2) + projections [128,1024]f32 (2).

All I/O in DRAM; per-core NEFF identical (SPMD over 8 cores).
"""

import os
import sys

if "/opt/trn_rl_repo" not in sys.path:
    sys.path.insert(0, "/opt/trn_rl_repo")
os.environ.setdefault("MYCRO_LOCAL_CACHE", "1")

import numpy as np
import ml_dtypes

import concourse.bass as bass
import concourse.bacc as bacc
import concourse.mybir as mybir
import concourse.tile as tile
from concourse.bass import ts

B, S, D, H, DK = 4, 2048, 1024, 16, 64
SQ = S // 2          # q rows per core
P = 128
NC = S // P          # 16 k-chunks
NJ = D // P          # 8 feature chunks
NP = H // 2          # 8 head pairs
DKE = DK + 1         # 65: vh head block + ones column
VW = H * DKE         # 1040
N_CORES = 8

F32 = mybir.dt.float32
BF16 = mybir.dt.bfloat16
AF = mybir.ActivationFunctionType


def build_program(n_iters=1):
    nc = bacc.Bacc(
        "TRN2",
        target_bir_lowering=False,
        debug=False,
        enable_asserts=False,
    )

    # ---- DRAM I/O (per-core slices; host pre-transposed, bf16) ----
    qT_d = nc.dram_tensor("qT", [D, SQ], BF16, kind="ExternalInput").ap()
    kT_d = nc.dram_tensor("kT", [D, S], BF16, kind="ExternalInput").ap()
    vT_d = nc.dram_tensor("vT", [D, S], BF16, kind="ExternalInput").ap()
    mT_d = nc.dram_tensor("maskT", [S, SQ], BF16, kind="ExternalInput").ap()
    wq_d = nc.dram_tensor("wq", [D, D], BF16, kind="ExternalInput").ap()
    wk_d = nc.dram_tensor("wk", [D, D], BF16, kind="ExternalInput").ap()
    wv_d = nc.dram_tensor("wv", [D, D], BF16, kind="ExternalInput").ap()
    wo_d = nc.dram_tensor("wo", [D, D], BF16, kind="ExternalInput").ap()
    # bias rows packed [4, D]: 0=bq (pre-scaled), 1=bk, 2=bv, 3=bo
    bias_d = nc.dram_tensor("biases", [4, D], BF16, kind="ExternalInput").ap()
    ones_d = nc.dram_tensor("ones_row", [1, SQ], BF16, kind="ExternalInput").ap()
    out_d = nc.dram_tensor("out", [SQ, D], F32, kind="ExternalOutput").ap()

    with tile.TileContext(nc) as tc:
        for _ in range(n_iters):
            _build(nc, tc, qT_d, kT_d, vT_d, mT_d,
                   wq_d, wk_d, wv_d, wo_d, bias_d, ones_d, out_d)

    nc.compile()
    return nc


def _build(nc, tc, qT_d, kT_d, vT_d, mT_d,
           wq_d, wk_d, wv_d, wo_d, bias_d, ones_d, out_d):
    from contextlib import ExitStack

    with ExitStack() as top:
        dram = top.enter_context(tc.tile_pool(name="dram", bufs=1, space="DRAM"))
        norm_dram = dram.tile([NP, 2, SQ], BF16)   # reciprocal denominators

        consts = top.enter_context(tc.tile_pool(name="consts", bufs=1))
        ones_row = consts.tile([1, SQ], BF16, tag="ones_row")
        nc.sync.dma_start(ones_row[:], ones_d)
        bq_row = consts.tile([1, D], BF16, tag="bq_row")
        nc.sync.dma_start(bq_row[:], bias_d[0:1, :])
        bk_row = consts.tile([1, D], BF16, tag="bk_row")
        nc.sync.dma_start(bk_row[:], bias_d[1:2, :])
        bv_row = consts.tile([1, D], BF16, tag="bv_row")
        nc.sync.dma_start(bv_row[:], bias_d[2:3, :])
        bo_row = consts.tile([1, D], BF16, tag="bo_row")
        nc.sync.dma_start(bo_row[:], bias_d[3:4, :])

        # persistent SBUF tensors
        inq = top.enter_context(tc.tile_pool(name="inq", bufs=1))
        qT_s = [inq.tile([P, SQ], BF16, tag=f"qT{d}", name=f"qT{d}")
                for d in range(NJ)]
        ink = top.enter_context(tc.tile_pool(name="ink", bufs=1))
        kT_s = [ink.tile([P, S], BF16, tag=f"kT{d}", name=f"kT{d}")
                for d in range(NJ)]
        kv = top.enter_context(tc.tile_pool(name="kv", bufs=1))
        vh = [kv.tile([P, VW], BF16, tag=f"vh{c}", name=f"vh{c}")
              for c in range(NC)]
        mk = [kv.tile([P, SQ], BF16, tag=f"mk{c}", name=f"mk{c}")
              for c in range(NC)]
        ctp = top.enter_context(tc.tile_pool(name="ctp", bufs=1))
        ctT = [ctp.tile([P, SQ], BF16, tag=f"ct{d}", name=f"ct{d}")
               for d in range(NJ)]
        wop = top.enter_context(tc.tile_pool(name="wop", bufs=1))
        wo_t = [wop.tile([P, D], BF16, tag=f"wo{d}", name=f"wo{d}")
                for d in range(NJ)]

        # ---------- Phase V: v projection -> vh (resident) ----------
        with ExitStack() as ctx:
            vin = ctx.enter_context(tc.tile_pool(name="vin", bufs=1))
            vT_s = [vin.tile([P, S], BF16, tag=f"vT{d}", name=f"vT{d}")
                    for d in range(NJ)]
            wvp = ctx.enter_context(tc.tile_pool(name="wvp", bufs=1))
            wv_t = [wvp.tile([P, D], BF16, tag=f"wv{d}", name=f"wv{d}")
                    for d in range(NJ)]
            psV = ctx.enter_context(
                tc.tile_pool(name="psV", bufs=2, space="PSUM"))
            for d in range(NJ):
                nc.sync.dma_start(wv_t[d][:], wv_d[ts(d, P), :])
                nc.sync.dma_start(vT_s[d][:], vT_d[ts(d, P), :])
            for c in range(NC):
                pv = psV.tile([P, D], F32, tag="pv")
                for h2 in range(D // 512):
                    # bias: pv[r, n] = bv[n] (stationary ones, moving bias)
                    nc.tensor.matmul(pv[:, ts(h2, 512)], ones_row[:, 0:P],
                                     bv_row[:, ts(h2, 512)],
                                     start=True, stop=False)
                    for d in range(NJ):
                        nc.tensor.matmul(
                            pv[:, ts(h2, 512)],
                            vT_s[d][:, ts(c, P)],
                            wv_t[d][:, ts(h2, 512)],
                            start=False, stop=(d == NJ - 1),
                        )
                # ones columns (written once; disjoint from the copy below)
                nc.vector.memset(
                    vh[c].rearrange("p (h w) -> p h w", w=DKE)[:, :, DK:DKE],
                    1.0)
                nc.vector.tensor_copy(
                    vh[c].rearrange("p (h w) -> p h w", w=DKE)[:, :, 0:DK],
                    pv[:].rearrange("p (h w) -> p h w", w=DK),
                )

        # mask loads (needed by all pairs)
        for c in range(NC):
            nc.sync.dma_start(mk[c][:], mT_d[ts(c, P), :])
        for d in range(NJ):
            nc.sync.dma_start(qT_s[d][:], qT_d[ts(d, P), :])
            nc.sync.dma_start(kT_s[d][:], kT_d[ts(d, P), :])
            nc.sync.dma_start(wo_t[d][:], wo_d[ts(d, P), :])

        # ---------- Main phase: per-pair projections + attention ----------
        with ExitStack() as ctx:
            wsl = ctx.enter_context(tc.tile_pool(name="wsl", bufs=2))
            qh_p = ctx.enter_context(tc.tile_pool(name="qh", bufs=2))
            kh_p = ctx.enter_context(tc.tile_pool(name="kh", bufs=2))
            ep = ctx.enter_context(tc.tile_pool(name="ep", bufs=3))
            mp = ctx.enter_context(tc.tile_pool(name="mp", bufs=3))
            rp = ctx.enter_context(tc.tile_pool(name="rp", bufs=1))
            ps_s = ctx.enter_context(
                tc.tile_pool(name="ps_s", bufs=2, space="PSUM"))
            ps_a = ctx.enter_context(
                tc.tile_pool(name="ps_a", bufs=1, space="PSUM"))
            psP = ctx.enter_context(
                tc.tile_pool(name="psP", bufs=1, space="PSUM"))

            for hp in range(NP):
                # -- weight slices for this pair's features --
                wqs = [wsl.tile([P, P], BF16, tag=f"wqs{d}", name=f"wqs{d}")
                       for d in range(NJ)]
                wks = [wsl.tile([P, P], BF16, tag=f"wks{d}", name=f"wks{d}")
                       for d in range(NJ)]
                for d in range(NJ):
                    nc.sync.dma_start(wqs[d][:], wq_d[ts(d, P), ts(hp, P)])
                    nc.sync.dma_start(wks[d][:], wk_d[ts(d, P), ts(hp, P)])

                # -- q projection (features hp*128..): qhT [128, SQ] --
                qhT = qh_p.tile([P, SQ], BF16, tag="qhT")
                pq = psP.tile([P, SQ], F32, tag="pp")
                for h2 in range(SQ // 512):
                    nc.tensor.matmul(pq[:, ts(h2, 512)],
                                     bq_row[:, ts(hp, P)],
                                     ones_row[:, ts(h2, 512)],
                                     start=True, stop=False)
                    for d in range(NJ):
                        nc.tensor.matmul(pq[:, ts(h2, 512)], wqs[d][:],
                                         qT_s[d][:, ts(h2, 512)],
                                         start=False, stop=(d == NJ - 1))
                nc.vector.tensor_copy(qhT[:], pq[:])

                # -- k projection: khT [128, S] in 2 halves --
                khT = kh_p.tile([P, S], BF16, tag="khT")
                for half in range(2):
                    pk = psP.tile([P, SQ], F32, tag="pp")
                    for h2 in range(SQ // 512):
                        nc.tensor.matmul(pk[:, ts(h2, 512)],
                                         bk_row[:, ts(hp, P)],
                                         ones_row[:, ts(h2, 512)],
                                         start=True, stop=False)
                        for d in range(NJ):
                            nc.tensor.matmul(
                                pk[:, ts(h2, 512)], wks[d][:],
                                kT_s[d][:, half * SQ + h2 * 512:
                                        half * SQ + (h2 + 1) * 512],
                                start=False, stop=(d == NJ - 1))
                    nc.vector.tensor_copy(khT[:, ts(half, SQ)], pk[:])

                # -- attention for the two heads of this pair --
                un_pair = rp.tile([P, SQ], BF16, tag="un")
                sums = [rp.tile([1, SQ], BF16, tag=f"sums{i}",
                                name=f"sums{i}") for i in range(2)]
                rsum = [rp.tile([1, SQ], BF16, tag=f"rsum{i}",
                                name=f"rsum{i}") for i in range(2)]
                for hh in range(2):
                    h = 2 * hp + hh
                    pa = ps_a.tile([DKE, SQ], F32, tag="pa")
                    for c in range(NC):
                        pscr = ps_s.tile([P, SQ], F32, tag="pscr")
                        for h2 in range(SQ // 512):
                            nc.tensor.matmul(
                                pscr[:, ts(h2, 512)],
                                khT[ts(hh, DK), ts(c, P)],
                                qhT[ts(hh, DK), ts(h2, 512)],
                                start=True, stop=True,
                            )
                        et = ep.tile([P, SQ], BF16, tag="et")
                        nc.scalar.activation(et[:], pscr[:], AF.Exp)
                        mt = mp.tile([P, SQ], BF16, tag="mt")
                        nc.vector.tensor_mul(mt[:], et[:], mk[c][:])
                        for h2 in range(SQ // 512):
                            nc.tensor.matmul(
                                pa[:, ts(h2, 512)],
                                vh[c][:, h * DKE:(h + 1) * DKE],
                                mt[:, ts(h2, 512)],
                                start=(c == 0), stop=(c == NC - 1),
                            )
                    # evacuate: unnormalized out + denominator row
                    nc.vector.tensor_copy(sums[hh][:], pa[DK:DKE, :])
                    nc.vector.tensor_copy(un_pair[ts(hh, DK), :], pa[0:DK, :])
                # normalize both heads with one broadcast multiply
                with nc.allow_low_precision("bf16 softmax denom reciprocal"):
                    nc.vector.reciprocal(rsum[0][:], sums[0][:])
                    nc.vector.reciprocal(rsum[1][:], sums[1][:])
                nc.sync.dma_start(
                    norm_dram[hp, 0].rearrange("(o n) -> o n", o=1), rsum[0][:])
                nc.sync.dma_start(
                    norm_dram[hp, 1].rearrange("(o n) -> o n", o=1), rsum[1][:])
                rb = rp.tile([P, SQ], BF16, tag="rb")
                for hh in range(2):
                    nc.sync.dma_start(
                        rb[ts(hh, DK), :],
                        norm_dram[hp, hh].rearrange("(o n) -> o n", o=1)
                        .to_broadcast((DK, SQ)))
                nc.vector.tensor_mul(ctT[hp][:], un_pair[:], rb[:])

            # ---------- output projection ----------
            stg = ctx.enter_context(tc.tile_pool(name="stg", bufs=2))
            for qt in range(NJ):
                po = psP.tile([P, D], F32, tag="pp")
                for h2 in range(D // 512):
                    nc.tensor.matmul(po[:, ts(h2, 512)], ones_row[:, 0:P],
                                     bo_row[:, ts(h2, 512)],
                                     start=True, stop=False)
                    for d in range(NJ):
                        nc.tensor.matmul(po[:, ts(h2, 512)],
                                         ctT[d][:, ts(qt, P)],
                                         wo_t[d][:, ts(h2, 512)],
                                         start=False, stop=(d == NJ - 1))
                st = stg.tile([P, D], F32, tag="sto")
                nc.scalar.activation(st[:], po[:], AF.Copy)
                nc.sync.dma_start(out_d[ts(qt, P), :], st[:])


def make_in_maps(q, k, v, att_mask):
    """Build the 8 per-core input dicts from full inputs (bf16 casts)."""
    bf = ml_dtypes.bfloat16
    q = np.asarray(q, dtype=np.float32)
    k = np.asarray(k, dtype=np.float32)
    v = np.asarray(v, dtype=np.float32)
    att_mask = np.asarray(att_mask)
    in_maps = []
    kT_b = [np.ascontiguousarray(k[b].T).astype(bf) for b in range(B)]
    vT_b = [np.ascontiguousarray(v[b].T).astype(bf) for b in range(B)]
    for c in range(N_CORES):
        b, half = divmod(c, 2)
        qs = slice(half * SQ, (half + 1) * SQ)
        in_maps.append({
            "qT": np.ascontiguousarray(q[b, qs, :].T).astype(bf),
            "kT": kT_b[b],
            "vT": vT_b[b],
            "maskT": np.ascontiguousarray(
                att_mask[b, qs, :].T).astype(bf),
        })
    return in_maps


def make_weights(W_q, b_q, W_k, b_k, W_v, b_v, W_o, b_o):
    """Shared per-core weight tensors (bf16; W_q/b_q pre-scaled 1/8)."""
    bf = ml_dtypes.bfloat16
    scale = 1.0 / np.sqrt(DK)
    biases = np.stack([
        np.asarray(b_q, np.float32) * scale,
        np.asarray(b_k, np.float32),
        np.asarray(b_v, np.float32),
        np.asarray(b_o, np.float32),
    ]).astype(bf)
    return {
        "wq": (np.asarray(W_q, np.float32) * scale).astype(bf),
        "wk": np.asarray(W_k, np.float32).astype(bf),
        "wv": np.asarray(W_v, np.float32).astype(bf),
        "wo": np.asarray(W_o, np.float32).astype(bf),
        "biases": biases,
        "ones_row": np.ones((1, SQ), dtype=bf),
    }


_PROG = None


def _get_program():
    global _PROG
    if _PROG is None:
        _PROG = build_program()
    return _PROG


def kernel(q, k, v, att_mask, W_q, b_q, W_k, b_k, W_v, b_v, W_o, b_o,
           **_ignored):
    from concourse.bass_utils import run_bass_kernel_spmd

    nc = _get_program()
    weights = make_weights(W_q, b_q, W_k, b_k, W_v, b_v, W_o, b_o)
    in_maps = [dict(m, **weights) for m in make_in_maps(q, k, v, att_mask)]
    res = run_bass_kernel_spmd(nc, in_maps, core_ids=list(range(N_CORES)))
    out = np.empty((B, S, D), dtype=np.float32)
    for c in range(N_CORES):
        b, half = divmod(c, 2)
        out[b, half * SQ:(half + 1) * SQ, :] = res.results[c]["out"]
    return out


# revision 41
# speedup vs baseline: 2.2141x; 2.2141x over previous
"""Multi-head attention TRN2 Bass kernel (v2, all-bf16).

Problem: B=4, S=2048, D=1024, H=16 heads (DK=64), fp32 reference, random
0/1 attention mask broadcast over heads.

Sharding: 8 cores = (batch b, query-half) pairs.  Core c handles batch
c//2, query rows [(c%2)*1024, (c%2+1)*1024).  K/V projections for the
batch are computed redundantly on the 2 cores sharing a batch; no
collectives, each core writes a disjoint output slice.

v2 design (vs v1 fp32r baseline):
  - Everything bf16: host pre-casts inputs/weights (W_q, b_q pre-scaled
    by 1/sqrt(DK) so no on-device scaling), intermediates bf16.  Matmul
    throughput is the same 1 col/cycle as fp32r, but bf16 halves SBUF/
    DMA, enables fast weight loads, N=1024 moving operands, and DVE
    2x mode for the mask multiply.
  - No DRAM spills: qhT/khT/vh/mask/ct all SBUF-resident (bf16 fits).
  - Scores matmuls for the two heads of a pair auto-pack into PE array
    row-halves (K=DK=64, tile_position (0,0)/(64,0) derived from base
    partitions), so consecutive head matmuls can overlap.
  - exp on ACT (PSUM fp32 -> SBUF bf16); this is the true bottleneck
    (~293us of ACT time per core); everything else overlaps it.
  - mask applied multiplicatively on DVE in bf16 2x mode.
  - AV matmul with 65-wide lhsT ([vh_h | 1]): row 64 accumulates the
    softmax denominators for free.
  - Normalization: denominators reciprocal'd once per head-pair,
    DMA-broadcast via DRAM round trip, one [128, SQ] bf16 2x multiply.
  - PSUM budget exactly 8 banks: scores 2x[128,1024]f32 (4) +
    attention accum [65,1024]f32 (2) + projections [128,1024]f32 (2).

All I/O in DRAM; per-core NEFF identical (SPMD over 8 cores).
"""

import os
import sys

if "/opt/trn_rl_repo" not in sys.path:
    sys.path.insert(0, "/opt/trn_rl_repo")
os.environ.setdefault("MYCRO_LOCAL_CACHE", "1")

import numpy as np
import ml_dtypes

import concourse.bass as bass
import concourse.bacc as bacc
import concourse.mybir as mybir
import concourse.tile as tile
from concourse.bass import ts

B, S, D, H, DK = 4, 2048, 1024, 16, 64
SQ = S // 2          # q rows per core
P = 128
NC = S // P          # 16 k-chunks
NJ = D // P          # 8 feature chunks
NP = H // 2          # 8 head pairs
DKE = DK + 1         # 65: vh head block + ones column
VW = H * DKE         # 1040
N_CORES = 8

F32 = mybir.dt.float32
BF16 = mybir.dt.bfloat16
AF = mybir.ActivationFunctionType


def build_program(n_iters=1):
    nc = bacc.Bacc(
        "TRN2",
        target_bir_lowering=False,
        debug=False,
        enable_asserts=False,
    )

    # ---- DRAM I/O (per-core slices; host pre-transposed, bf16) ----
    qT_d = nc.dram_tensor("qT", [D, SQ], BF16, kind="ExternalInput").ap()
    kT_d = nc.dram_tensor("kT", [D, S], BF16, kind="ExternalInput").ap()
    vT_d = nc.dram_tensor("vT", [D, S], BF16, kind="ExternalInput").ap()
    mT_d = nc.dram_tensor("maskT", [S, SQ], BF16, kind="ExternalInput").ap()
    wq_d = nc.dram_tensor("wq", [D, D], BF16, kind="ExternalInput").ap()
    wk_d = nc.dram_tensor("wk", [D, D], BF16, kind="ExternalInput").ap()
    wv_d = nc.dram_tensor("wv", [D, D], BF16, kind="ExternalInput").ap()
    wo_d = nc.dram_tensor("wo", [D, D], BF16, kind="ExternalInput").ap()
    # bias rows packed [4, D]: 0=bq (pre-scaled), 1=bk, 2=bv, 3=bo
    bias_d = nc.dram_tensor("biases", [4, D], BF16, kind="ExternalInput").ap()
    ones_d = nc.dram_tensor("ones_row", [1, SQ], BF16, kind="ExternalInput").ap()
    out_d = nc.dram_tensor("out", [SQ, D], F32, kind="ExternalOutput").ap()

    with tile.TileContext(nc) as tc:
        for _ in range(n_iters):
            _build(nc, tc, qT_d, kT_d, vT_d, mT_d,
                   wq_d, wk_d, wv_d, wo_d, bias_d, ones_d, out_d)

    nc.compile()
    return nc


def _build(nc, tc, qT_d, kT_d, vT_d, mT_d,
           wq_d, wk_d, wv_d, wo_d, bias_d, ones_d, out_d):
    from contextlib import ExitStack

    with ExitStack() as top:
        dram = top.enter_context(tc.tile_pool(name="dram", bufs=1, space="DRAM"))
        norm_dram = dram.tile([NP, 2, SQ], BF16)   # reciprocal denominators

        consts = top.enter_context(tc.tile_pool(name="consts", bufs=1))
        ones_row = consts.tile([1, SQ], BF16, tag="ones_row")
        nc.sync.dma_start(ones_row[:], ones_d)
        bq_row = consts.tile([1, D], BF16, tag="bq_row")
        nc.sync.dma_start(bq_row[:], bias_d[0:1, :])
        bk_row = consts.tile([1, D], BF16, tag="bk_row")
        nc.sync.dma_start(bk_row[:], bias_d[1:2, :])
        bv_row = consts.tile([1, D], BF16, tag="bv_row")
        nc.sync.dma_start(bv_row[:], bias_d[2:3, :])
        bo_row = consts.tile([1, D], BF16, tag="bo_row")
        nc.sync.dma_start(bo_row[:], bias_d[3:4, :])

        # persistent SBUF tensors
        inq = top.enter_context(tc.tile_pool(name="inq", bufs=1))
        qT_s = [inq.tile([P, SQ], BF16, tag=f"qT{d}", name=f"qT{d}")
                for d in range(NJ)]
        ink = top.enter_context(tc.tile_pool(name="ink", bufs=1))
        kT_s = [ink.tile([P, S], BF16, tag=f"kT{d}", name=f"kT{d}")
                for d in range(NJ)]
        kv = top.enter_context(tc.tile_pool(name="kv", bufs=1))
        vh = [kv.tile([P, VW], BF16, tag=f"vh{c}", name=f"vh{c}")
              for c in range(NC)]
        mk = [kv.tile([P, SQ], BF16, tag=f"mk{c}", name=f"mk{c}")
              for c in range(NC)]
        ctp = top.enter_context(tc.tile_pool(name="ctp", bufs=1))
        ctT = [ctp.tile([P, SQ], BF16, tag=f"ct{d}", name=f"ct{d}")
               for d in range(NJ)]
        wop = top.enter_context(tc.tile_pool(name="wop", bufs=1))
        wo_t = [wop.tile([P, D], BF16, tag=f"wo{d}", name=f"wo{d}")
                for d in range(NJ)]

        # ---------- Phase V: v projection -> vh (resident) ----------
        with ExitStack() as vctx:
            vin = vctx.enter_context(tc.tile_pool(name="vin", bufs=1))
            vT_s = [vin.tile([P, S], BF16, tag=f"vT{d}", name=f"vT{d}")
                    for d in range(NJ)]
            wvp = vctx.enter_context(tc.tile_pool(name="wvp", bufs=1))
            wv_t = [wvp.tile([P, D], BF16, tag=f"wv{d}", name=f"wv{d}")
                    for d in range(NJ)]
            psV = vctx.enter_context(
                tc.tile_pool(name="psV", bufs=2, space="PSUM"))
            for d in range(NJ):
                nc.sync.dma_start(wv_t[d][:], wv_d[ts(d, P), :])
                nc.sync.dma_start(vT_s[d][:], vT_d[ts(d, P), :])
            for c in range(NC):
                pv = psV.tile([P, D], F32, tag="pv")
                for h2 in range(D // 512):
                    # bias: pv[r, n] = bv[n] (stationary ones, moving bias)
                    nc.tensor.matmul(pv[:, ts(h2, 512)], ones_row[:, 0:P],
                                     bv_row[:, ts(h2, 512)],
                                     start=True, stop=False)
                for d in range(NJ):
                    for h2 in range(D // 512):
                        nc.tensor.matmul(
                            pv[:, ts(h2, 512)],
                            vT_s[d][:, ts(c, P)],
                            wv_t[d][:, ts(h2, 512)],
                            start=False, stop=(d == NJ - 1),
                        )
                nc.vector.memset(
                    vh[c].rearrange("p (h w) -> p h w", w=DKE)[:, :, DK:DKE],
                    1.0)
                nc.vector.tensor_copy(
                    vh[c].rearrange("p (h w) -> p h w", w=DKE)[:, :, 0:DK],
                    pv[:].rearrange("p (h w) -> p h w", w=DK),
                )

        # mask + q/k input loads (needed from pair 0 onwards)
        for c in range(NC):
            nc.sync.dma_start(mk[c][:], mT_d[ts(c, P), :])
        for d in range(NJ):
            nc.sync.dma_start(qT_s[d][:], qT_d[ts(d, P), :])
            nc.sync.dma_start(kT_s[d][:], kT_d[ts(d, P), :])
            nc.sync.dma_start(wo_t[d][:], wo_d[ts(d, P), :])

        # ---------- Main phase: V + per-pair projections + attention ----
        with ExitStack() as ctx:
            wsl = ctx.enter_context(tc.tile_pool(name="wsl", bufs=2))
            qh_p = ctx.enter_context(tc.tile_pool(name="qh", bufs=2))
            kh_p = ctx.enter_context(tc.tile_pool(name="kh", bufs=2))
            ep = ctx.enter_context(tc.tile_pool(name="ep", bufs=3))
            mp = ctx.enter_context(tc.tile_pool(name="mp", bufs=3))
            rp = ctx.enter_context(tc.tile_pool(name="rp", bufs=1))
            ps_s = ctx.enter_context(
                tc.tile_pool(name="ps_s", bufs=2, space="PSUM"))
            ps_a = ctx.enter_context(
                tc.tile_pool(name="ps_a", bufs=1, space="PSUM"))
            psP = ctx.enter_context(
                tc.tile_pool(name="psP", bufs=1, space="PSUM"))

            def emit_proj(hp):
                # -- weight slices for this pair's features --
                wqs = [wsl.tile([P, P], BF16, tag=f"wqs{d}", name=f"wqs{d}")
                       for d in range(NJ)]
                wks = [wsl.tile([P, P], BF16, tag=f"wks{d}", name=f"wks{d}")
                       for d in range(NJ)]
                for d in range(NJ):
                    nc.sync.dma_start(wqs[d][:], wq_d[ts(d, P), ts(hp, P)])
                    nc.sync.dma_start(wks[d][:], wk_d[ts(d, P), ts(hp, P)])

                # -- q projection (features hp*128..): qhT [128, SQ] --
                qhT = qh_p.tile([P, SQ], BF16, tag="qhT")
                pq = psP.tile([P, SQ], F32, tag="pp")
                for h2 in range(SQ // 512):
                    nc.tensor.matmul(pq[:, ts(h2, 512)],
                                     bq_row[:, ts(hp, P)],
                                     ones_row[:, ts(h2, 512)],
                                     start=True, stop=False)
                for d in range(NJ):
                    for h2 in range(SQ // 512):
                        nc.tensor.matmul(pq[:, ts(h2, 512)], wqs[d][:],
                                         qT_s[d][:, ts(h2, 512)],
                                         start=False, stop=(d == NJ - 1))
                nc.vector.tensor_copy(qhT[:], pq[:])

                # -- k projection: khT [128, S] in 2 halves --
                khT = kh_p.tile([P, S], BF16, tag="khT")
                for half in range(2):
                    pk = psP.tile([P, SQ], F32, tag="pp")
                    for h2 in range(SQ // 512):
                        nc.tensor.matmul(pk[:, ts(h2, 512)],
                                         bk_row[:, ts(hp, P)],
                                         ones_row[:, ts(h2, 512)],
                                         start=True, stop=False)
                    for d in range(NJ):
                        for h2 in range(SQ // 512):
                            nc.tensor.matmul(
                                pk[:, ts(h2, 512)], wks[d][:],
                                kT_s[d][:, half * SQ + h2 * 512:
                                        half * SQ + (h2 + 1) * 512],
                                start=False, stop=(d == NJ - 1))
                    nc.vector.tensor_copy(khT[:, ts(half, SQ)], pk[:])
                return qhT, khT

            def emit_attention(hp, qhT, khT):
                un65 = [rp.tile([DKE, SQ], BF16, tag=f"un{i}",
                                name=f"un{i}") for i in range(2)]
                rsum = [rp.tile([1, SQ], BF16, tag=f"rsum{i}",
                                name=f"rsum{i}") for i in range(2)]
                for hh in range(2):
                    h = 2 * hp + hh
                    pa = ps_a.tile([DKE, SQ], F32, tag="pa")
                    for c in range(NC):
                        pscr = ps_s.tile([P, SQ], F32, tag="pscr")
                        for h2 in range(SQ // 512):
                            nc.tensor.matmul(
                                pscr[:, ts(h2, 512)],
                                khT[ts(hh, DK), ts(c, P)],
                                qhT[ts(hh, DK), ts(h2, 512)],
                                start=True, stop=True,
                            )
                        et = ep.tile([P, SQ], BF16, tag="et")
                        nc.scalar.activation(et[:], pscr[:], AF.Exp)
                        mt = mp.tile([P, SQ], BF16, tag="mt")
                        nc.vector.tensor_mul(mt[:], et[:], mk[c][:])
                        for h2 in range(SQ // 512):
                            nc.tensor.matmul(
                                pa[:, ts(h2, 512)],
                                vh[c][:, h * DKE:(h + 1) * DKE],
                                mt[:, ts(h2, 512)],
                                start=(c == 0), stop=(c == NC - 1),
                            )
                    # evacuate unnormalized out + denominator row (row 64)
                    nc.vector.tensor_copy(un65[hh][:], pa[0:DKE, :])
                    with nc.allow_low_precision("bf16 softmax denom recip"):
                        nc.vector.reciprocal(rsum[hh][:], un65[hh][DK:DKE, :])
                    nc.sync.dma_start(
                        norm_dram[hp, hh].rearrange("(o n) -> o n", o=1),
                        rsum[hh][:])
                # normalize both heads via DMA-broadcast reciprocal rows
                for hh in range(2):
                    rb = rp.tile([DK, SQ], BF16, tag=f"rb{hh}",
                                 name=f"rb{hh}")
                    nc.sync.dma_start(
                        rb[:],
                        norm_dram[hp, hh].rearrange("(o n) -> o n", o=1)
                        .to_broadcast((DK, SQ)))
                    nc.vector.tensor_mul(ctT[hp][ts(hh, DK), :],
                                         un65[hh][0:DK, :], rb[:])

            qk = emit_proj(0)
            for hp in range(NP):
                nxt = emit_proj(hp + 1) if hp + 1 < NP else None
                emit_attention(hp, *qk)
                qk = nxt

            # ---------- output projection ----------
            stg = ctx.enter_context(tc.tile_pool(name="stg", bufs=2))
            for qt in range(NJ):
                po = psP.tile([P, D], F32, tag="pp")
                for h2 in range(D // 512):
                    nc.tensor.matmul(po[:, ts(h2, 512)], ones_row[:, 0:P],
                                     bo_row[:, ts(h2, 512)],
                                     start=True, stop=False)
                for d in range(NJ):
                    for h2 in range(D // 512):
                        nc.tensor.matmul(po[:, ts(h2, 512)],
                                         ctT[d][:, ts(qt, P)],
                                         wo_t[d][:, ts(h2, 512)],
                                         start=False, stop=(d == NJ - 1))
                st = stg.tile([P, D], F32, tag="sto")
                nc.scalar.activation(st[:], po[:], AF.Copy)
                nc.sync.dma_start(out_d[ts(qt, P), :], st[:])


def make_in_maps(q, k, v, att_mask):
    """Build the 8 per-core input dicts from full inputs (bf16 casts)."""
    bf = ml_dtypes.bfloat16
    q = np.asarray(q, dtype=np.float32)
    k = np.asarray(k, dtype=np.float32)
    v = np.asarray(v, dtype=np.float32)
    att_mask = np.asarray(att_mask)
    in_maps = []
    kT_b = [np.ascontiguousarray(k[b].T).astype(bf) for b in range(B)]
    vT_b = [np.ascontiguousarray(v[b].T).astype(bf) for b in range(B)]
    for c in range(N_CORES):
        b, half = divmod(c, 2)
        qs = slice(half * SQ, (half + 1) * SQ)
        in_maps.append({
            "qT": np.ascontiguousarray(q[b, qs, :].T).astype(bf),
            "kT": kT_b[b],
            "vT": vT_b[b],
            "maskT": np.ascontiguousarray(
                att_mask[b, qs, :].T).astype(bf),
        })
    return in_maps


def make_weights(W_q, b_q, W_k, b_k, W_v, b_v, W_o, b_o):
    """Shared per-core weight tensors (bf16; W_q/b_q pre-scaled 1/8)."""
    bf = ml_dtypes.bfloat16
    scale = 1.0 / np.sqrt(DK)
    biases = np.stack([
        np.asarray(b_q, np.float32) * scale,
        np.asarray(b_k, np.float32),
        np.asarray(b_v, np.float32),
        np.asarray(b_o, np.float32),
    ]).astype(bf)
    return {
        "wq": (np.asarray(W_q, np.float32) * scale).astype(bf),
        "wk": np.asarray(W_k, np.float32).astype(bf),
        "wv": np.asarray(W_v, np.float32).astype(bf),
        "wo": np.asarray(W_o, np.float32).astype(bf),
        "biases": biases,
        "ones_row": np.ones((1, SQ), dtype=bf),
    }


_PROG = None


def _get_program():
    global _PROG
    if _PROG is None:
        _PROG = build_program()
    return _PROG


def kernel(q, k, v, att_mask, W_q, b_q, W_k, b_k, W_v, b_v, W_o, b_o,
           **_ignored):
    from concourse.bass_utils import run_bass_kernel_spmd

    nc = _get_program()
    weights = make_weights(W_q, b_q, W_k, b_k, W_v, b_v, W_o, b_o)
    in_maps = [dict(m, **weights) for m in make_in_maps(q, k, v, att_mask)]
    res = run_bass_kernel_spmd(nc, in_maps, core_ids=list(range(N_CORES)))
    out = np.empty((B, S, D), dtype=np.float32)
    for c in range(N_CORES):
        b, half = divmod(c, 2)
        out[b, half * SQ:(half + 1) * SQ, :] = res.results[c]["out"]
    return out
